# revision 1
# baseline (speedup 1.0000x reference)
"""Trainium2 Bass kernel for nn_GroupPointEncoder.

Reference computation (G=4, B=8, N=2048, F=128):
  std = 2 or 4 per point by label class
  coords = [point_coord, (point_coord + noise*std)[1:]]           # [G,B,N,3]
  normed = (coords - low) / (high - low)
  pe     = interleaved sin/cos embedding, (y,x,z) order            # [G,B,N,384]
  h      = relu(pe @ W1.T + b1)                                    # [G,B,N,512]
  pos    = h @ W2.T + b2                                           # [G,B,N,256]
  query  = label_weight[labels] + pos
  out    = concat([query_pos, query], -1).reshape(G*B, N, 512)

Sharding: data-parallel over the G*B=32 (g,b) pairs, 4 per core, 8 cores.
Each core computes its 4*2048=8192 points' `query` half on device; the
query_pos half is a passthrough assembled on the host.

Device layout (feature-major): per 512-point tile
  arg[128,3,512](PSUM)  = outer(s_k, prescaled_coords)   3 K=1 matmuls
  range-reduce arg to [-pi,pi] via int32 round-trip      DVE/GpSimd
  pe[128,3,512]         = Sin(arg + bias_vec)            1 ACT op (bias 0 / pi/2)
  h[128,4,512]          = relu(W1p @ pe + b1)            12 f32r matmuls + DVE
  q[128,2,512]          = W2 @ h + onehot.T@(lab_w+b2)   10 f32r matmuls accum
"""
import sys
import math

sys.path.insert(0, "/opt/trn_rl_repo")

import numpy as np
from contextlib import ExitStack

import concourse.bass as bass
import concourse.tile as tile
from concourse import bacc, library_config, mybir
from concourse.bass_utils import run_bass_kernel_spmd

# problem constants (hardcoded per contract)
G, B, N, F = 4, 8, 2048, 128
NCORES = 8
BPC = B * G // NCORES          # 4 (g,b) pairs per core
NPTS = BPC * N                 # 8192 points per core
T = 512                        # points per tile
NT = NPTS // T                 # 16 tiles
TWO_PI = 2.0 * math.pi
INV_TWO_PI = 1.0 / TWO_PI
F32 = mybir.dt.float32
F32R = mybir.dt.float32r
I32 = mybir.dt.int32

_CACHE = {}


def _build_program():
    nc = bacc.Bacc("TRN2", target_bir_lowering=False, debug=False, num_devices=NCORES)

    pc_d = nc.dram_tensor("pc", [NT, 1, 3, T], F32, kind="ExternalInput").ap()
    oh_d = nc.dram_tensor("oh", [NT, 10, T], F32R, kind="ExternalInput").ap()
    w1t_d = nc.dram_tensor("w1t", [3, 128, 512], F32R, kind="ExternalInput").ap()
    w2t_d = nc.dram_tensor("w2t", [4, 128, 256], F32R, kind="ExternalInput").ap()
    lwb_d = nc.dram_tensor("lwb", [10, 256], F32R, kind="ExternalInput").ap()
    svec_d = nc.dram_tensor("svec", [128, 1], F32, kind="ExternalInput").ap()
    sdiv_d = nc.dram_tensor("sdiv", [128, 1], F32, kind="ExternalInput").ap()
    invs2_d = nc.dram_tensor("invs2", [128, 1], F32, kind="ExternalInput").ap()
    bvec_d = nc.dram_tensor("bvec", [128, 1], F32, kind="ExternalInput").ap()
    b1c_d = nc.dram_tensor("b1c", [128, 4], F32, kind="ExternalInput").ap()
    q_d = nc.dram_tensor("q", [256, NPTS], F32, kind="ExternalOutput").ap()

    with tile.TileContext(nc) as tc, ExitStack() as ctx:
        cpool = ctx.enter_context(tc.tile_pool(name="consts", bufs=1))
        wpool = ctx.enter_context(tc.tile_pool(name="weights", bufs=1))
        io = ctx.enter_context(tc.tile_pool(name="io", bufs=3))
        work = ctx.enter_context(tc.tile_pool(name="work", bufs=2))
        psum_h = ctx.enter_context(tc.tile_pool(name="ph", bufs=1, space="PSUM"))
        psum_q = ctx.enter_context(tc.tile_pool(name="pq", bufs=2, space="PSUM"))

        nc.gpsimd.load_library(library_config.proxy)
        svec = cpool.tile([128, 1], F32)
        nc.sync.dma_start(svec[:], svec_d[:])
        sdiv = cpool.tile([128, 1], F32)
        nc.sync.dma_start(sdiv[:], sdiv_d[:])
        invs2 = cpool.tile([128, 1], F32)
        nc.sync.dma_start(invs2[:], invs2_d[:])
        bvec = cpool.tile([128, 1], F32)
        nc.sync.dma_start(bvec[:], bvec_d[:])
        b1c = cpool.tile([128, 4], F32)
        nc.sync.dma_start(b1c[:], b1c_d[:])
        lwb = cpool.tile([10, 256], F32R)
        nc.sync.dma_start(lwb[:], lwb_d[:])

        w1t = []
        for k in range(3):
            w = wpool.tile([128, 512], F32R, name=f"w1t{k}", tag=f"w1t{k}")
            nc.sync.dma_start(w[:], w1t_d[k])
            w1t.append(w)
        w2t = []
        for k in range(4):
            w = wpool.tile([128, 256], F32R, name=f"w2t{k}", tag=f"w2t{k}")
            nc.sync.dma_start(w[:], w2t_d[k])
            w2t.append(w)

        for t in range(NT):
            pc_t = io.tile([1, 3, T], F32, tag="pc_t")
            nc.sync.dma_start(pc_t[:], pc_d[t])
            oh_t = io.tile([10, T], F32R, tag="oh_t")
            nc.sync.dma_start(oh_t[:], oh_d[t])

            # ---- stage 1: broadcast prescaled coords across partitions (exact f32)
            bc = work.tile([128, 3, T], F32, tag="bc")
            for c in range(3):
                nc.gpsimd.partition_broadcast(bc[:, c, :], pc_t[:, c, :])

            # ---- stage 2: phase reduction in coordinate space:
            #   ki = round(bc * s/2pi);  bc2 = bc - ki * 2pi/s
            # then arg = s*bc2 = s*bc - 2pi*ki  lands in [-pi, pi]
            ki = work.tile([128, 3, T], I32, tag="ki")
            nc.vector.tensor_scalar(ki[:], bc[:], sdiv[:], None, op0=mybir.AluOpType.mult)
            kf = work.tile([128, 3, T], F32, tag="kf")
            nc.vector.tensor_scalar(kf[:], ki[:], invs2[:], None, op0=mybir.AluOpType.mult)
            bc2 = work.tile([128, 3, T], F32, tag="bc2")
            nc.gpsimd.tensor_sub(bc2[:], bc[:], kf[:])

            # ---- stage 3: pe = sin(s*bc2 + bias)  (rows 0:64 sin, 64:128 cos)
            pe = work.tile([128, 3, T], F32R, tag="pe")
            nc.scalar.activation(
                pe[:],
                bc2[:],
                mybir.ActivationFunctionType.Sin,
                bias=bvec[:],
                scale=svec[:],
            )

            # ---- stage 4: h = relu(W1p @ pe + b1), feature-major [4x128, T]
            hp = psum_h.tile([128, 4, T], F32, tag="hp")
            for m in range(4):
                for k in range(3):
                    nc.tensor.matmul(
                        hp[:, m, :],
                        w1t[k][:, m * 128 : (m + 1) * 128],
                        pe[:, k, :],
                        start=(k == 0),
                        stop=(k == 2),
                    )
            h = work.tile([128, 4, T], F32R, tag="h")
            for m in range(4):
                nc.scalar.activation(
                    h[:, m, :],
                    hp[:, m, :],
                    mybir.ActivationFunctionType.Relu,
                    bias=b1c[:, m : m + 1],
                )

            # ---- stage 5: q = W2 @ h + onehot^T-gather, feature-major [2x128, T]
            for mp in range(2):
                qp = psum_q.tile([128, T], F32, tag="qp")
                for k in range(4):
                    nc.tensor.matmul(
                        qp[:],
                        w2t[k][:, mp * 128 : (mp + 1) * 128],
                        h[:, k, :],
                        start=(k == 0),
                        stop=False,
                    )
                nc.tensor.matmul(
                    qp[:],
                    lwb[:, mp * 128 : (mp + 1) * 128],
                    oh_t[:],
                    start=False,
                    stop=True,
                )
                qs = work.tile([128, T], F32, tag="qs")
                nc.vector.tensor_copy(qs[:], qp[:])
                nc.sync.dma_start(q_d[mp * 128 : (mp + 1) * 128, t * T : (t + 1) * T], qs[:])

    nc.compile()
    return nc


def _host_prep(point_coord, labels, pc_range, noise, label_weight, W1, b1, W2, b2):
    """Build the per-core input maps (host-side sharding + weight prep)."""
    pc32 = np.asarray(point_coord, np.float32)
    lab = np.asarray(labels)
    noi = np.asarray(noise, np.float32)
    rng = np.asarray(pc_range, np.float32)

    small = (lab == 0) | (lab >= 6)
    std = np.where(small, 2.0, 4.0).astype(np.float32)            # [B,N]
    coords = pc32[None] + noi * std[None, :, :, None]             # [G,B,N,3]
    coords[0] = pc32                                              # group 0 originals
    low, high = rng[:3], rng[3:]
    pcs = (coords - low) / (high - low) * np.float32(TWO_PI)      # [G,B,N,3]
    pcs = pcs[..., [1, 0, 2]]   # reference concatenates pe in (y,x,z) order
    onehot = np.eye(10, dtype=np.float32)[np.asarray(lab, np.int64)]  # [B,N,10]

    # feature permutation: kernel row c*128+k -> ref feature c*128+2k (sin),
    # row c*128+64+k -> c*128+2k+1 (cos)
    perm = np.empty(3 * F, np.int64)
    for c in range(3):
        for k in range(64):
            perm[c * 128 + k] = c * 128 + 2 * k
            perm[c * 128 + 64 + k] = c * 128 + 2 * k + 1
    w1p = np.ascontiguousarray(np.asarray(W1, np.float32)[:, perm].T)  # [384,512]
    w2t = np.ascontiguousarray(np.asarray(W2, np.float32).T)           # [512,256]
    lwb = np.asarray(label_weight, np.float32) + np.asarray(b2, np.float32)[None]
    b1c = np.ascontiguousarray(np.asarray(b1, np.float32).reshape(4, 128).T)

    k64 = np.arange(64, dtype=np.float64)
    s64 = 10000.0 ** (-k64 / 64.0)
    s128 = np.concatenate([s64, s64])
    svec = s128.astype(np.float32).reshape(128, 1)
    sdiv = (s128 / (2 * np.pi)).astype(np.float32).reshape(128, 1)
    invs2 = (2 * np.pi / s128).astype(np.float32).reshape(128, 1)
    bvec = np.concatenate(
        [np.zeros(64, np.float32), np.full(64, np.pi / 2, np.float32)]
    ).reshape(128, 1)

    shared = {
        "w1t": w1p.reshape(3, 128, 512),
        "w2t": w2t.reshape(4, 128, 256),
        "lwb": np.ascontiguousarray(lwb),
        "svec": np.ascontiguousarray(svec),
        "sdiv": np.ascontiguousarray(sdiv),
        "invs2": np.ascontiguousarray(invs2),
        "bvec": np.ascontiguousarray(bvec),
        "b1c": b1c,
    }

    in_maps = []
    for core in range(NCORES):
        g = core // 2
        b0 = 4 * (core % 2)
        # [4b, N, 3] -> [3, NPTS] -> [3, NT, T] -> [NT, 3, T]
        pcc = pcs[g, b0 : b0 + 4].reshape(NPTS, 3).T
        pcc = np.ascontiguousarray(pcc.reshape(3, NT, T).transpose(1, 0, 2)).reshape(
            NT, 1, 3, T
        )
        ohc = onehot[b0 : b0 + 4].reshape(NPTS, 10).T
        ohc = np.ascontiguousarray(ohc.reshape(10, NT, T).transpose(1, 0, 2))
        in_maps.append({"pc": pcc, "oh": ohc, **shared})
    return in_maps


def _get_nc():
    if "nc" not in _CACHE:
        _CACHE["nc"] = _build_program()
    return _CACHE["nc"]


def _run_device(in_maps, trace=False, **kw):
    nc = _get_nc()
    return run_bass_kernel_spmd(nc, in_maps, list(range(NCORES)), trace=trace, **kw)


def kernel(point_coord, labels, pc_range, noise, query_pos, label_weight, W1, b1, W2, b2):
    in_maps = _host_prep(
        point_coord, labels, pc_range, noise, label_weight, W1, b1, W2, b2
    )
    res = _run_device(in_maps)

    qp = np.asarray(query_pos, np.float32)
    out = np.empty((G * B, N, 4 * F), np.float32)
    out[:, :, : 2 * F] = qp.reshape(G * B, N, 2 * F)
    for core in range(NCORES):
        q = res.results[core]["q"]                       # [256, NPTS]
        q = q.reshape(2 * F, BPC, N).transpose(1, 2, 0)  # [4, N, 256]
        out[4 * core : 4 * core + 4, :, 2 * F :] = q
    return out



# revision 5
# speedup vs baseline: 1.1543x; 1.1543x over previous
"""Trainium2 Bass kernel for nn_GroupPointEncoder.

Reference computation (G=4, B=8, N=2048, F=128):
  std = 2 or 4 per point by label class
  coords = [point_coord, (point_coord + noise*std)[1:]]           # [G,B,N,3]
  normed = (coords - low) / (high - low)
  pe     = interleaved sin/cos embedding, (y,x,z) order            # [G,B,N,384]
  h      = relu(pe @ W1.T + b1)                                    # [G,B,N,512]
  pos    = h @ W2.T + b2                                           # [G,B,N,256]
  query  = label_weight[labels] + pos
  out    = concat([query_pos, query], -1).reshape(G*B, N, 512)

Sharding: data-parallel over the G*B=32 (g,b) pairs, 4 per core, 8 cores.
Each core computes its 4*2048=8192 points' `query` half on device; the
query_pos half is a passthrough assembled on the host.

Device pipeline (feature-major, per 512-point tile):
  bc[128,3,512]   = DMA partition-broadcast of prescaled coords (f32)
  k  (int32)      = round(bc*s/2pi + b/2pi)          DVE tensor_scalar
  r               = bc - k*(2pi/s)  (Cody-Waite)     DVE custom op
  pe (bf16)       = Sin(r*s + b)                     1 ACT op, arg in [-pi,pi]
  h  (bf16)       = relu(W1p @ pe + b1)              12 bf16 matmuls + 4 ACT
  q  (f32)        = W2 @ h + onehot.T@(lab_w+b2)     10 bf16 matmuls
                    PSUM -> SBUF copies on GpSimd, 1 output DMA per tile
"""
import sys
import math

sys.path.insert(0, "/opt/trn_rl_repo")

import numpy as np
import ml_dtypes
from contextlib import ExitStack

import concourse.bass as bass
import concourse.tile as tile
from concourse import bacc, library_config, mybir
from concourse.bass_utils import run_bass_kernel_spmd

# problem constants (hardcoded per contract)
G, B, N, F = 4, 8, 2048, 128
NCORES = 8
BPC = B * G // NCORES          # 4 (g,b) pairs per core
NPTS = BPC * N                 # 8192 points per core
T = 512                        # points per tile
NT = NPTS // T                 # 16 tiles
TWO_PI = 2.0 * math.pi
F32 = mybir.dt.float32
BF16 = mybir.dt.bfloat16
I32 = mybir.dt.int32
BF16_NP = ml_dtypes.bfloat16

_CACHE = {}


def _build_program():
    nc = bacc.Bacc("TRN2", target_bir_lowering=False, debug=False, num_devices=NCORES)

    pc_d = nc.dram_tensor("pc", [NT, 1, 3, T], F32, kind="ExternalInput").ap()
    oh_d = nc.dram_tensor("oh", [NT, 10, T], BF16, kind="ExternalInput").ap()
    w1t_d = nc.dram_tensor("w1t", [3, 128, 512], BF16, kind="ExternalInput").ap()
    w2t_d = nc.dram_tensor("w2t", [4, 128, 256], BF16, kind="ExternalInput").ap()
    lwb_d = nc.dram_tensor("lwb", [10, 256], BF16, kind="ExternalInput").ap()
    svec_d = nc.dram_tensor("svec", [128, 1], F32, kind="ExternalInput").ap()
    bvec_d = nc.dram_tensor("bvec", [128, 1], F32, kind="ExternalInput").ap()
    sdiv_d = nc.dram_tensor("sdiv", [128, 1], F32, kind="ExternalInput").ap()
    bdiv_d = nc.dram_tensor("bdiv", [128, 1], F32, kind="ExternalInput").ap()
    ivhi_d = nc.dram_tensor("ivhi", [128, 1], F32, kind="ExternalInput").ap()
    ivmid_d = nc.dram_tensor("ivmid", [128, 1], F32, kind="ExternalInput").ap()
    b1c_d = nc.dram_tensor("b1c", [128, 4], F32, kind="ExternalInput").ap()
    q_d = nc.dram_tensor("q", [128, 2, NPTS], F32, kind="ExternalOutput").ap()

    with tile.TileContext(nc) as tc, ExitStack() as ctx:
        cpool = ctx.enter_context(tc.tile_pool(name="consts", bufs=1))
        wpool = ctx.enter_context(tc.tile_pool(name="weights", bufs=1))
        io = ctx.enter_context(tc.tile_pool(name="io", bufs=3))
        bcp = ctx.enter_context(tc.tile_pool(name="bcp", bufs=2))
        work = ctx.enter_context(tc.tile_pool(name="work", bufs=2))
        psum_h = ctx.enter_context(tc.tile_pool(name="ph", bufs=1, space="PSUM"))
        psum_q = ctx.enter_context(tc.tile_pool(name="pq", bufs=2, space="PSUM"))

        nc.gpsimd.load_library(library_config.proxy)
        svec = cpool.tile([128, 1], F32)
        nc.sync.dma_start(svec[:], svec_d[:])
        bvec = cpool.tile([128, 1], F32)
        nc.sync.dma_start(bvec[:], bvec_d[:])
        sdiv = cpool.tile([128, 1], F32)
        nc.sync.dma_start(sdiv[:], sdiv_d[:])
        bdiv = cpool.tile([128, 1], F32)
        nc.sync.dma_start(bdiv[:], bdiv_d[:])
        ivhi = cpool.tile([128, 1], F32)
        nc.sync.dma_start(ivhi[:], ivhi_d[:])
        ivmid = cpool.tile([128, 1], F32)
        nc.sync.dma_start(ivmid[:], ivmid_d[:])
        b1c = cpool.tile([128, 4], F32)
        nc.sync.dma_start(b1c[:], b1c_d[:])
        lwb = cpool.tile([10, 256], BF16)
        nc.sync.dma_start(lwb[:], lwb_d[:])

        w1t = []
        for kk in range(3):
            w = wpool.tile([128, 512], BF16, name=f"w1t{kk}", tag=f"w1t{kk}")
            nc.sync.dma_start(w[:], w1t_d[kk])
            w1t.append(w)
        w2t = []
        for kk in range(4):
            w = wpool.tile([128, 256], BF16, name=f"w2t{kk}", tag=f"w2t{kk}")
            nc.sync.dma_start(w[:], w2t_d[kk])
            w2t.append(w)

        for t in range(NT):
            # ---- stage 0: DMA broadcast coords to all 128 partitions
            bc = bcp.tile([128, 3, T], F32, tag="bc")
            nc.sync.dma_start(bc[:], pc_d[t].to_broadcast((128, 3, T)))
            oh_t = io.tile([10, T], BF16, tag="oh_t")
            nc.sync.dma_start(oh_t[:], oh_d[t])

            # ---- stage 1: range reduction.  k = round(bc*s/2pi + b/2pi);
            # r = bc - k*(2pi/s) via Cody-Waite; then s*r + b lands in [-pi,pi]
            bcf = bc[:].rearrange("p a b -> p (a b)")
            k = work.tile([128, 3 * T], I32, tag="k")
            nc.gpsimd.tensor_scalar(
                k[:], bcf, sdiv[:], bdiv[:],
                op0=mybir.AluOpType.mult, op1=mybir.AluOpType.add,
            )
            r = work.tile([128, 3 * T], F32, tag="r")
            nc.vector.cody_waite_cascade(r[:], bcf, k[:], ivhi[:], ivmid[:], 0.0)

            # ---- stage 2: pe = sin(s*r + b)  (rows 0:64 sin, 64:128 cos)
            pe = work.tile([128, 3, T], BF16, tag="pe")
            nc.scalar.activation(
                pe[:].rearrange("p a b -> p (a b)"), r[:],
                mybir.ActivationFunctionType.Sin,
                bias=bvec[:], scale=svec[:],
            )

            # ---- stage 3: h = relu(W1p @ pe + b1), feature-major [4x128, T]
            hp = psum_h.tile([128, 4, T], F32, tag="hp")
            for m in range(4):
                for kk in range(3):
                    nc.tensor.matmul(
                        hp[:, m, :],
                        w1t[kk][:, m * 128 : (m + 1) * 128],
                        pe[:, kk, :],
                        start=(kk == 0),
                        stop=(kk == 2),
                    )
            h = work.tile([128, 4, T], BF16, tag="h")
            for m in range(4):
                nc.scalar.activation(
                    h[:, m, :],
                    hp[:, m, :],
                    mybir.ActivationFunctionType.Relu,
                    bias=b1c[:, m : m + 1],
                )

            # ---- stage 4: q = W2 @ h + onehot-gather, feature-major [2x128, T]
            qs = work.tile([128, 2, T], F32, tag="qs")
            for mp in range(2):
                qp = psum_q.tile([128, T], F32, tag="qp")
                for kk in range(4):
                    nc.tensor.matmul(
                        qp[:],
                        w2t[kk][:, mp * 128 : (mp + 1) * 128],
                        h[:, kk, :],
                        start=(kk == 0),
                        stop=False,
                    )
                nc.tensor.matmul(
                    qp[:],
                    lwb[:, mp * 128 : (mp + 1) * 128],
                    oh_t[:],
                    start=False,
                    stop=True,
                )
                nc.vector.tensor_copy(qs[:, mp, :], qp[:])
            nc.sync.dma_start(q_d[:, :, t * T : (t + 1) * T], qs[:])

    nc.compile()
    return nc


def _host_prep(point_coord, labels, pc_range, noise, label_weight, W1, b1, W2, b2):
    """Build the per-core input maps (host-side sharding + weight prep)."""
    pc32 = np.asarray(point_coord, np.float32)
    lab = np.asarray(labels)
    noi = np.asarray(noise, np.float32)
    rng = np.asarray(pc_range, np.float32)

    small = (lab == 0) | (lab >= 6)
    std = np.where(small, 2.0, 4.0).astype(np.float32)            # [B,N]
    coords = pc32[None] + noi * std[None, :, :, None]             # [G,B,N,3]
    coords[0] = pc32                                              # group 0 originals
    low, high = rng[:3], rng[3:]
    pcs = (coords - low) / (high - low) * np.float32(TWO_PI)      # [G,B,N,3]
    pcs = pcs[..., [1, 0, 2]]   # reference concatenates pe in (y,x,z) order
    onehot = np.eye(10, dtype=np.float32)[np.asarray(lab, np.int64)]  # [B,N,10]

    # feature permutation: kernel row c*128+j -> ref feature c*128+2j (sin),
    # row c*128+64+j -> c*128+2j+1 (cos)
    perm = np.empty(3 * F, np.int64)
    for c in range(3):
        for j in range(64):
            perm[c * 128 + j] = c * 128 + 2 * j
            perm[c * 128 + 64 + j] = c * 128 + 2 * j + 1
    w1p = np.ascontiguousarray(np.asarray(W1, np.float32)[:, perm].T)  # [384,512]
    w2t = np.ascontiguousarray(np.asarray(W2, np.float32).T)           # [512,256]
    lwb = np.asarray(label_weight, np.float32) + np.asarray(b2, np.float32)[None]
    b1c = np.ascontiguousarray(np.asarray(b1, np.float32).reshape(4, 128).T)

    j64 = np.arange(64, dtype=np.float64)
    s64 = 10000.0 ** (-j64 / 64.0)
    s128 = np.concatenate([s64, s64])
    b128 = np.concatenate([np.zeros(64), np.full(64, np.pi / 2)])
    inv = 2 * np.pi / s128                                         # f64
    ivhi = inv.astype(np.float32).view(np.uint32) & np.uint32(0xFFFFE000)
    ivhi = ivhi.view(np.float32)          # 10 explicit mantissa bits: k*ivhi exact
    ivmid = (inv - ivhi.astype(np.float64)).astype(np.float32)

    def col(v):
        return np.ascontiguousarray(v.astype(np.float32).reshape(128, 1))

    shared = {
        "w1t": w1p.astype(BF16_NP).reshape(3, 128, 512),
        "w2t": w2t.astype(BF16_NP).reshape(4, 128, 256),
        "lwb": np.ascontiguousarray(lwb.astype(BF16_NP)),
        "svec": col(s128),
        "bvec": col(b128),
        "sdiv": col(s128 / (2 * np.pi)),
        "bdiv": col(b128 / (2 * np.pi)),
        "ivhi": col(ivhi),
        "ivmid": col(ivmid),
        "b1c": b1c,
    }

    in_maps = []
    for core in range(NCORES):
        g = core // 2
        b0 = 4 * (core % 2)
        # [4b, N, 3] -> [3, NPTS] -> [3, NT, T] -> [NT, 3, T]
        pcc = pcs[g, b0 : b0 + 4].reshape(NPTS, 3).T
        pcc = np.ascontiguousarray(pcc.reshape(3, NT, T).transpose(1, 0, 2)).reshape(
            NT, 1, 3, T
        )
        ohc = onehot[b0 : b0 + 4].reshape(NPTS, 10).T
        ohc = np.ascontiguousarray(
            ohc.reshape(10, NT, T).transpose(1, 0, 2).astype(BF16_NP)
        )
        in_maps.append({"pc": pcc, "oh": ohc, **shared})
    return in_maps


def _get_nc():
    if "nc" not in _CACHE:
        _CACHE["nc"] = _build_program()
    return _CACHE["nc"]


def _run_device(in_maps, trace=False, **kw):
    nc = _get_nc()
    return run_bass_kernel_spmd(nc, in_maps, list(range(NCORES)), trace=trace, **kw)


def kernel(point_coord, labels, pc_range, noise, query_pos, label_weight, W1, b1, W2, b2):
    in_maps = _host_prep(
        point_coord, labels, pc_range, noise, label_weight, W1, b1, W2, b2
    )
    res = _run_device(in_maps)

    qp = np.asarray(query_pos, np.float32)
    out = np.empty((G * B, N, 4 * F), np.float32)
    out[:, :, : 2 * F] = qp.reshape(G * B, N, 2 * F)
    for core in range(NCORES):
        q3 = res.results[core]["q"]                      # [128, 2, NPTS]
        q = q3.transpose(1, 0, 2).reshape(2 * F, BPC, N)  # [256, 4, N]
        out[4 * core : 4 * core + 4, :, 2 * F :] = q.transpose(1, 2, 0)
    return out


# revision 8
# speedup vs baseline: 1.8066x; 1.5651x over previous
"""Trainium2 Bass kernel for nn_GroupPointEncoder.

Reference computation (G=4, B=8, N=2048, F=128):
  std = 2 or 4 per point by label class
  coords = [point_coord, (point_coord + noise*std)[1:]]           # [G,B,N,3]
  normed = (coords - low) / (high - low)
  pe     = interleaved sin/cos embedding, (y,x,z) order            # [G,B,N,384]
  h      = relu(pe @ W1.T + b1)                                    # [G,B,N,512]
  pos    = h @ W2.T + b2                                           # [G,B,N,256]
  query  = label_weight[labels] + pos
  out    = concat([query_pos, query], -1).reshape(G*B, N, 512)

Sharding: data-parallel over the G*B=32 (g,b) pairs, 4 per core, 8 cores.
Each core computes its 4*2048=8192 points' `query` half on device; the
query_pos half is a passthrough assembled on the host.

Device pipeline (feature-major, per 512-point tile):
  bc[128,3,512]   = DMA partition-broadcast of prescaled coords (f32)
  k  (int32)      = round(bc*s/2pi + b/2pi)          DVE tensor_scalar
  r               = bc - k*(2pi/s)  (Cody-Waite)     DVE custom op
  pe (bf16)       = Sin(r*s + b)                     1 ACT op, arg in [-pi,pi]
  h  (bf16)       = relu(W1p @ pe + b1)              12 bf16 matmuls + 4 ACT
  q  (f32)        = W2 @ h + onehot.T@(lab_w+b2)     10 bf16 matmuls
                    PSUM -> SBUF copies on GpSimd, 1 output DMA per tile
"""
import sys
import math

sys.path.insert(0, "/opt/trn_rl_repo")

import numpy as np
import ml_dtypes
from contextlib import ExitStack

import concourse.bass as bass
import concourse.tile as tile
from concourse import bacc, library_config, mybir
from concourse.bass_utils import run_bass_kernel_spmd

# problem constants (hardcoded per contract)
G, B, N, F = 4, 8, 2048, 128
NCORES = 8
BPC = B * G // NCORES          # 4 (g,b) pairs per core
NPTS = BPC * N                 # 8192 points per core
T = 512                        # points per tile
NT = NPTS // T                 # 16 tiles
TWO_PI = 2.0 * math.pi
F32 = mybir.dt.float32
BF16 = mybir.dt.bfloat16
I32 = mybir.dt.int32
BF16_NP = ml_dtypes.bfloat16

_CACHE = {}


def _build_program():
    nc = bacc.Bacc("TRN2", target_bir_lowering=False, debug=False, num_devices=NCORES)

    pc_d = nc.dram_tensor("pc", [NT, 1, 3, T], F32, kind="ExternalInput").ap()
    oh_d = nc.dram_tensor("oh", [NT, 10, T], BF16, kind="ExternalInput").ap()
    w1t_d = nc.dram_tensor("w1t", [3, 128, 512], BF16, kind="ExternalInput").ap()
    w2t_d = nc.dram_tensor("w2t", [4, 128, 256], BF16, kind="ExternalInput").ap()
    lwb_d = nc.dram_tensor("lwb", [10, 256], BF16, kind="ExternalInput").ap()
    svec_d = nc.dram_tensor("svec", [128, 1], F32, kind="ExternalInput").ap()
    bvec_d = nc.dram_tensor("bvec", [128, 1], F32, kind="ExternalInput").ap()
    sdiv_d = nc.dram_tensor("sdiv", [128, 1], F32, kind="ExternalInput").ap()
    bdiv_d = nc.dram_tensor("bdiv", [128, 1], F32, kind="ExternalInput").ap()
    ivhi_d = nc.dram_tensor("ivhi", [128, 1], F32, kind="ExternalInput").ap()
    ivmid_d = nc.dram_tensor("ivmid", [128, 1], F32, kind="ExternalInput").ap()
    b1c_d = nc.dram_tensor("b1c", [128, 4], F32, kind="ExternalInput").ap()
    q_d = nc.dram_tensor("q", [128, 2, NPTS], F32, kind="ExternalOutput").ap()

    with tile.TileContext(nc) as tc, ExitStack() as ctx:
        cpool = ctx.enter_context(tc.tile_pool(name="consts", bufs=1))
        wpool = ctx.enter_context(tc.tile_pool(name="weights", bufs=1))
        io = ctx.enter_context(tc.tile_pool(name="io", bufs=4))
        bcp = ctx.enter_context(tc.tile_pool(name="bcp", bufs=4))
        work = ctx.enter_context(tc.tile_pool(name="work", bufs=2))
        qsp = ctx.enter_context(tc.tile_pool(name="qsp", bufs=3))
        psum_h = ctx.enter_context(tc.tile_pool(name="ph", bufs=3, space="PSUM"))
        psum_q = ctx.enter_context(tc.tile_pool(name="pq", bufs=2, space="PSUM"))

        nc.gpsimd.load_library(library_config.proxy)
        svec = cpool.tile([128, 1], F32)
        nc.sync.dma_start(svec[:], svec_d[:])
        bvec = cpool.tile([128, 1], F32)
        nc.sync.dma_start(bvec[:], bvec_d[:])
        sdiv = cpool.tile([128, 1], F32)
        nc.sync.dma_start(sdiv[:], sdiv_d[:])
        bdiv = cpool.tile([128, 1], F32)
        nc.sync.dma_start(bdiv[:], bdiv_d[:])
        ivhi = cpool.tile([128, 1], F32)
        nc.sync.dma_start(ivhi[:], ivhi_d[:])
        ivmid = cpool.tile([128, 1], F32)
        nc.sync.dma_start(ivmid[:], ivmid_d[:])
        b1c = cpool.tile([128, 4], F32)
        nc.sync.dma_start(b1c[:], b1c_d[:])
        lwb = cpool.tile([10, 256], BF16)
        nc.sync.dma_start(lwb[:], lwb_d[:])

        w1t = []
        for kk in range(3):
            w = wpool.tile([128, 512], BF16, name=f"w1t{kk}", tag=f"w1t{kk}")
            nc.sync.dma_start(w[:], w1t_d[kk])
            w1t.append(w)
        w2t = []
        for kk in range(4):
            w = wpool.tile([128, 256], BF16, name=f"w2t{kk}", tag=f"w2t{kk}")
            nc.sync.dma_start(w[:], w2t_d[kk])
            w2t.append(w)

        # input prefetch runs 2 tiles ahead; output DMA issue is skewed 1 tile
        # behind compute so the sync queue never head-of-line blocks on it
        bc_tiles, oh_tiles, out_pend = {}, {}, {}

        def _prefetch(t):
            if t >= NT:
                return
            bc_ = bcp.tile([128, 3, T], F32, tag="bc")
            nc.sync.dma_start(bc_[:], pc_d[t].to_broadcast((128, 3, T)))
            oh_ = io.tile([10, T], BF16, tag="oh_t")
            nc.sync.dma_start(oh_[:], oh_d[t])
            bc_tiles[t], oh_tiles[t] = bc_, oh_

        _prefetch(0)
        _prefetch(1)
        for t in range(NT):
            _prefetch(t + 2)
            if t - 1 in out_pend:
                nc.sync.dma_start(
                    q_d[:, :, (t - 1) * T : t * T], out_pend.pop(t - 1)[:]
                )
            bc = bc_tiles.pop(t)
            oh_t = oh_tiles.pop(t)

            # ---- stage 1: range reduction.  k = round(bc*s/2pi + b/2pi);
            # r = bc - k*(2pi/s) via Cody-Waite; then s*r + b lands in [-pi,pi]
            bcf = bc[:].rearrange("p a b -> p (a b)")
            k = work.tile([128, 3 * T], I32, tag="k")
            nc.gpsimd.tensor_scalar(
                k[:], bcf, sdiv[:], bdiv[:],
                op0=mybir.AluOpType.mult, op1=mybir.AluOpType.add,
            )
            r = work.tile([128, 3 * T], F32, tag="r")
            nc.vector.cody_waite_cascade(r[:], bcf, k[:], ivhi[:], ivmid[:], 0.0)

            # ---- stage 2: pe = sin(s*r + b)  (rows 0:64 sin, 64:128 cos)
            pe = work.tile([128, 3, T], BF16, tag="pe")
            nc.scalar.activation(
                pe[:].rearrange("p a b -> p (a b)"), r[:],
                mybir.ActivationFunctionType.Sin,
                bias=bvec[:], scale=svec[:],
            )

            # ---- stage 3: h = relu(W1p @ pe + b1), feature-major [4x128, T]
            # two PSUM half-tiles so next tile's matmuls never wait on relu
            h = work.tile([128, 4, T], BF16, tag="h")
            for half in range(2):
                hp = psum_h.tile([128, 2, T], F32, tag="hp")
                for m2 in range(2):
                    m = 2 * half + m2
                    for kk in range(3):
                        nc.tensor.matmul(
                            hp[:, m2, :],
                            w1t[kk][:, m * 128 : (m + 1) * 128],
                            pe[:, kk, :],
                            start=(kk == 0),
                            stop=(kk == 2),
                        )
                    nc.scalar.activation(
                        h[:, m, :],
                        hp[:, m2, :],
                        mybir.ActivationFunctionType.Relu,
                        bias=b1c[:, m : m + 1],
                    )

            # ---- stage 4: q = W2 @ h + onehot-gather, feature-major [2x128, T]
            qs = qsp.tile([128, 2, T], F32, tag="qs")
            for mp in range(2):
                qp = psum_q.tile([128, T], F32, tag="qp")
                for kk in range(4):
                    nc.tensor.matmul(
                        qp[:],
                        w2t[kk][:, mp * 128 : (mp + 1) * 128],
                        h[:, kk, :],
                        start=(kk == 0),
                        stop=False,
                    )
                nc.tensor.matmul(
                    qp[:],
                    lwb[:, mp * 128 : (mp + 1) * 128],
                    oh_t[:],
                    start=False,
                    stop=True,
                )
                nc.vector.tensor_copy(qs[:, mp, :], qp[:])
            out_pend[t] = qs
        nc.sync.dma_start(q_d[:, :, (NT - 1) * T :], out_pend.pop(NT - 1)[:])

    nc.compile()
    return nc


def _host_prep(point_coord, labels, pc_range, noise, label_weight, W1, b1, W2, b2):
    """Build the per-core input maps (host-side sharding + weight prep)."""
    pc32 = np.asarray(point_coord, np.float32)
    lab = np.asarray(labels)
    noi = np.asarray(noise, np.float32)
    rng = np.asarray(pc_range, np.float32)

    small = (lab == 0) | (lab >= 6)
    std = np.where(small, 2.0, 4.0).astype(np.float32)            # [B,N]
    coords = pc32[None] + noi * std[None, :, :, None]             # [G,B,N,3]
    coords[0] = pc32                                              # group 0 originals
    low, high = rng[:3], rng[3:]
    pcs = (coords - low) / (high - low) * np.float32(TWO_PI)      # [G,B,N,3]
    pcs = pcs[..., [1, 0, 2]]   # reference concatenates pe in (y,x,z) order
    onehot = np.eye(10, dtype=np.float32)[np.asarray(lab, np.int64)]  # [B,N,10]

    # feature permutation: kernel row c*128+j -> ref feature c*128+2j (sin),
    # row c*128+64+j -> c*128+2j+1 (cos)
    perm = np.empty(3 * F, np.int64)
    for c in range(3):
        for j in range(64):
            perm[c * 128 + j] = c * 128 + 2 * j
            perm[c * 128 + 64 + j] = c * 128 + 2 * j + 1
    w1p = np.ascontiguousarray(np.asarray(W1, np.float32)[:, perm].T)  # [384,512]
    w2t = np.ascontiguousarray(np.asarray(W2, np.float32).T)           # [512,256]
    lwb = np.asarray(label_weight, np.float32) + np.asarray(b2, np.float32)[None]
    b1c = np.ascontiguousarray(np.asarray(b1, np.float32).reshape(4, 128).T)

    j64 = np.arange(64, dtype=np.float64)
    s64 = 10000.0 ** (-j64 / 64.0)
    s128 = np.concatenate([s64, s64])
    b128 = np.concatenate([np.zeros(64), np.full(64, np.pi / 2)])
    inv = 2 * np.pi / s128                                         # f64
    ivhi = inv.astype(np.float32).view(np.uint32) & np.uint32(0xFFFFE000)
    ivhi = ivhi.view(np.float32)          # 10 explicit mantissa bits: k*ivhi exact
    ivmid = (inv - ivhi.astype(np.float64)).astype(np.float32)

    def col(v):
        return np.ascontiguousarray(v.astype(np.float32).reshape(128, 1))

    shared = {
        "w1t": w1p.astype(BF16_NP).reshape(3, 128, 512),
        "w2t": w2t.astype(BF16_NP).reshape(4, 128, 256),
        "lwb": np.ascontiguousarray(lwb.astype(BF16_NP)),
        "svec": col(s128),
        "bvec": col(b128),
        "sdiv": col(s128 / (2 * np.pi)),
        "bdiv": col(b128 / (2 * np.pi)),
        "ivhi": col(ivhi),
        "ivmid": col(ivmid),
        "b1c": b1c,
    }

    in_maps = []
    for core in range(NCORES):
        g = core // 2
        b0 = 4 * (core % 2)
        # [4b, N, 3] -> [3, NPTS] -> [3, NT, T] -> [NT, 3, T]
        pcc = pcs[g, b0 : b0 + 4].reshape(NPTS, 3).T
        pcc = np.ascontiguousarray(pcc.reshape(3, NT, T).transpose(1, 0, 2)).reshape(
            NT, 1, 3, T
        )
        ohc = onehot[b0 : b0 + 4].reshape(NPTS, 10).T
        ohc = np.ascontiguousarray(
            ohc.reshape(10, NT, T).transpose(1, 0, 2).astype(BF16_NP)
        )
        in_maps.append({"pc": pcc, "oh": ohc, **shared})
    return in_maps


def _get_nc():
    if "nc" not in _CACHE:
        _CACHE["nc"] = _build_program()
    return _CACHE["nc"]


def _run_device(in_maps, trace=False, **kw):
    nc = _get_nc()
    return run_bass_kernel_spmd(nc, in_maps, list(range(NCORES)), trace=trace, **kw)


def kernel(point_coord, labels, pc_range, noise, query_pos, label_weight, W1, b1, W2, b2):
    in_maps = _host_prep(
        point_coord, labels, pc_range, noise, label_weight, W1, b1, W2, b2
    )
    res = _run_device(in_maps)

    qp = np.asarray(query_pos, np.float32)
    out = np.empty((G * B, N, 4 * F), np.float32)
    out[:, :, : 2 * F] = qp.reshape(G * B, N, 2 * F)
    for core in range(NCORES):
        q3 = res.results[core]["q"]                      # [128, 2, NPTS]
        q = q3.transpose(1, 0, 2).reshape(2 * F, BPC, N)  # [256, 4, N]
        out[4 * core : 4 * core + 4, :, 2 * F :] = q.transpose(1, 2, 0)
    return out


# revision 10
# speedup vs baseline: 1.8326x; 1.0144x over previous
"""Trainium2 Bass kernel for nn_GroupPointEncoder.

Reference computation (G=4, B=8, N=2048, F=128):
  std = 2 or 4 per point by label class
  coords = [point_coord, (point_coord + noise*std)[1:]]           # [G,B,N,3]
  normed = (coords - low) / (high - low)
  pe     = interleaved sin/cos embedding, (y,x,z) order            # [G,B,N,384]
  h      = relu(pe @ W1.T + b1)                                    # [G,B,N,512]
  pos    = h @ W2.T + b2                                           # [G,B,N,256]
  query  = label_weight[labels] + pos
  out    = concat([query_pos, query], -1).reshape(G*B, N, 512)

Sharding: data-parallel over the G*B=32 (g,b) pairs, 4 per core, 8 cores.
Each core computes its 4*2048=8192 points' `query` half on device; the
query_pos half is a passthrough assembled on the host.

Device pipeline (feature-major, per 512-point tile):
  bc[128,3,512]   = DMA partition-broadcast of prescaled coords (f32)
  k  (int32)      = round(bc*s/2pi + b/2pi)          DVE tensor_scalar
  r               = bc - k*(2pi/s)  (Cody-Waite)     DVE custom op
  pe (bf16)       = Sin(r*s + b)                     1 ACT op, arg in [-pi,pi]
  h  (bf16)       = relu(W1p @ pe + b1)              12 bf16 matmuls + 4 ACT
  q  (f32)        = W2 @ h + onehot.T@(lab_w+b2)     10 bf16 matmuls
                    PSUM -> SBUF copies on GpSimd, 1 output DMA per tile
"""
import sys
import math

sys.path.insert(0, "/opt/trn_rl_repo")

import numpy as np
import ml_dtypes
from contextlib import ExitStack

import concourse.bass as bass
import concourse.tile as tile
from concourse import bacc, library_config, mybir
from concourse.bass_utils import run_bass_kernel_spmd

# problem constants (hardcoded per contract)
G, B, N, F = 4, 8, 2048, 128
NCORES = 8
BPC = B * G // NCORES          # 4 (g,b) pairs per core
NPTS = BPC * N                 # 8192 points per core
T = 512                        # points per tile
NT = NPTS // T                 # 16 tiles
TWO_PI = 2.0 * math.pi
F32 = mybir.dt.float32
BF16 = mybir.dt.bfloat16
I32 = mybir.dt.int32
BF16_NP = ml_dtypes.bfloat16

_CACHE = {}


def _build_program():
    nc = bacc.Bacc("TRN2", target_bir_lowering=False, debug=False, num_devices=NCORES)

    pc_d = nc.dram_tensor("pc", [NT, 1, 3, T], F32, kind="ExternalInput").ap()
    oh_d = nc.dram_tensor("oh", [NT, 10, T], BF16, kind="ExternalInput").ap()
    w1t_d = nc.dram_tensor("w1t", [3, 128, 512], BF16, kind="ExternalInput").ap()
    w2t_d = nc.dram_tensor("w2t", [4, 128, 256], BF16, kind="ExternalInput").ap()
    lwb_d = nc.dram_tensor("lwb", [10, 256], BF16, kind="ExternalInput").ap()
    svec_d = nc.dram_tensor("svec", [128, 1], F32, kind="ExternalInput").ap()
    bvec_d = nc.dram_tensor("bvec", [128, 1], F32, kind="ExternalInput").ap()
    sdiv_d = nc.dram_tensor("sdiv", [128, 1], F32, kind="ExternalInput").ap()
    bdiv_d = nc.dram_tensor("bdiv", [128, 1], F32, kind="ExternalInput").ap()
    ivhi_d = nc.dram_tensor("ivhi", [128, 1], F32, kind="ExternalInput").ap()
    ivmid_d = nc.dram_tensor("ivmid", [128, 1], F32, kind="ExternalInput").ap()
    b1c_d = nc.dram_tensor("b1c", [128, 4], F32, kind="ExternalInput").ap()
    q_d = nc.dram_tensor("q", [128, 2, NPTS], F32, kind="ExternalOutput").ap()

    with tile.TileContext(nc) as tc, ExitStack() as ctx:
        cpool = ctx.enter_context(tc.tile_pool(name="consts", bufs=1))
        wpool = ctx.enter_context(tc.tile_pool(name="weights", bufs=1))
        io = ctx.enter_context(tc.tile_pool(name="io", bufs=4))
        bcp = ctx.enter_context(tc.tile_pool(name="bcp", bufs=4))
        work = ctx.enter_context(tc.tile_pool(name="work", bufs=2))
        qsp = ctx.enter_context(tc.tile_pool(name="qsp", bufs=3))
        psum_h = ctx.enter_context(tc.tile_pool(name="ph", bufs=3, space="PSUM"))
        psum_q = ctx.enter_context(tc.tile_pool(name="pq", bufs=2, space="PSUM"))

        nc.gpsimd.load_library(library_config.proxy)
        svec = cpool.tile([128, 1], F32)
        nc.sync.dma_start(svec[:], svec_d[:])
        bvec = cpool.tile([128, 1], F32)
        nc.sync.dma_start(bvec[:], bvec_d[:])
        sdiv = cpool.tile([128, 1], F32)
        nc.sync.dma_start(sdiv[:], sdiv_d[:])
        bdiv = cpool.tile([128, 1], F32)
        nc.sync.dma_start(bdiv[:], bdiv_d[:])
        ivhi = cpool.tile([128, 1], F32)
        nc.sync.dma_start(ivhi[:], ivhi_d[:])
        ivmid = cpool.tile([128, 1], F32)
        nc.sync.dma_start(ivmid[:], ivmid_d[:])
        b1c = cpool.tile([128, 4], F32)
        nc.sync.dma_start(b1c[:], b1c_d[:])
        lwb = cpool.tile([10, 256], BF16)
        nc.sync.dma_start(lwb[:], lwb_d[:])

        w1t = []
        for kk in range(3):
            w = wpool.tile([128, 512], BF16, name=f"w1t{kk}", tag=f"w1t{kk}")
            nc.sync.dma_start(w[:], w1t_d[kk])
            w1t.append(w)
        w2t = []
        for kk in range(4):
            w = wpool.tile([128, 256], BF16, name=f"w2t{kk}", tag=f"w2t{kk}")
            nc.sync.dma_start(w[:], w2t_d[kk])
            w2t.append(w)

        # input prefetch runs 2 tiles ahead; output DMA issue is skewed 1 tile
        # behind compute so the sync queue never head-of-line blocks on it
        bc_tiles, oh_tiles, out_pend = {}, {}, {}

        def _prefetch(t):
            if t >= NT:
                return
            bc_ = bcp.tile([128, 3, T], F32, tag="bc")
            nc.sync.dma_start(bc_[:], pc_d[t].to_broadcast((128, 3, T)))
            oh_ = io.tile([10, T], BF16, tag="oh_t")
            nc.sync.dma_start(oh_[:], oh_d[t])
            bc_tiles[t], oh_tiles[t] = bc_, oh_

        _prefetch(0)
        _prefetch(1)
        for t in range(NT):
            _prefetch(t + 2)
            if t - 1 in out_pend:
                nc.sync.dma_start(
                    q_d[:, :, (t - 1) * T : t * T], out_pend.pop(t - 1)[:]
                )
            bc = bc_tiles.pop(t)
            oh_t = oh_tiles.pop(t)

            # ---- stage 1: range reduction.  k = round(bc*s/2pi + b/2pi);
            # r = bc - k*(2pi/s) via Cody-Waite; then s*r + b lands in [-pi,pi]
            bcf = bc[:].rearrange("p a b -> p (a b)")
            k = work.tile([128, 3 * T], I32, tag="k")
            nc.gpsimd.tensor_scalar(
                k[:], bcf, sdiv[:], bdiv[:],
                op0=mybir.AluOpType.mult, op1=mybir.AluOpType.add,
            )
            r = work.tile([128, 3 * T], F32, tag="r")
            nc.vector.cody_waite_cascade(r[:], bcf, k[:], ivhi[:], ivmid[:], 0.0)

            # ---- stage 2: pe = sin(s*r + b)  (rows 0:64 sin, 64:128 cos)
            # 3 per-coordinate chunks so the first W1 matmul starts sooner
            pe = work.tile([128, 3, T], BF16, tag="pe")
            rv = r[:].rearrange("p (a b) -> p a b", a=3)
            for c in range(3):
                nc.scalar.activation(
                    pe[:, c, :], rv[:, c, :],
                    mybir.ActivationFunctionType.Sin,
                    bias=bvec[:], scale=svec[:],
                )

            # ---- stage 3: h = relu(W1p @ pe + b1), feature-major [4x128, T]
            # two PSUM half-tiles so next tile's matmuls never wait on relu;
            # relu m=1 runs on DVE so relu pairs complete in parallel
            h = work.tile([128, 4, T], BF16, tag="h")
            for half in range(2):
                hp = psum_h.tile([128, 2, T], F32, tag="hp")
                for m2 in range(2):
                    m = 2 * half + m2
                    for kk in range(3):
                        nc.tensor.matmul(
                            hp[:, m2, :],
                            w1t[kk][:, m * 128 : (m + 1) * 128],
                            pe[:, kk, :],
                            start=(kk == 0),
                            stop=(kk == 2),
                        )
                    if m == 1:
                        nc.vector.tensor_scalar(
                            h[:, m, :], hp[:, m2, :], b1c[:, m : m + 1], 0.0,
                            op0=mybir.AluOpType.add, op1=mybir.AluOpType.max,
                        )
                    else:
                        nc.scalar.activation(
                            h[:, m, :],
                            hp[:, m2, :],
                            mybir.ActivationFunctionType.Relu,
                            bias=b1c[:, m : m + 1],
                        )

            # ---- stage 4: q = W2 @ h + onehot-gather, feature-major [2x128, T]
            qs = qsp.tile([128, 2, T], F32, tag="qs")
            for mp in range(2):
                qp = psum_q.tile([128, T], F32, tag="qp")
                for kk in range(4):
                    nc.tensor.matmul(
                        qp[:],
                        w2t[kk][:, mp * 128 : (mp + 1) * 128],
                        h[:, kk, :],
                        start=(kk == 0),
                        stop=False,
                    )
                nc.tensor.matmul(
                    qp[:],
                    lwb[:, mp * 128 : (mp + 1) * 128],
                    oh_t[:],
                    start=False,
                    stop=True,
                )
                nc.vector.tensor_copy(qs[:, mp, :], qp[:])
            out_pend[t] = qs
        nc.sync.dma_start(q_d[:, :, (NT - 1) * T :], out_pend.pop(NT - 1)[:])

    nc.compile()
    return nc


def _host_prep(point_coord, labels, pc_range, noise, label_weight, W1, b1, W2, b2):
    """Build the per-core input maps (host-side sharding + weight prep)."""
    pc32 = np.asarray(point_coord, np.float32)
    lab = np.asarray(labels)
    noi = np.asarray(noise, np.float32)
    rng = np.asarray(pc_range, np.float32)

    small = (lab == 0) | (lab >= 6)
    std = np.where(small, 2.0, 4.0).astype(np.float32)            # [B,N]
    coords = pc32[None] + noi * std[None, :, :, None]             # [G,B,N,3]
    coords[0] = pc32                                              # group 0 originals
    low, high = rng[:3], rng[3:]
    pcs = (coords - low) / (high - low) * np.float32(TWO_PI)      # [G,B,N,3]
    pcs = pcs[..., [1, 0, 2]]   # reference concatenates pe in (y,x,z) order
    onehot = np.eye(10, dtype=np.float32)[np.asarray(lab, np.int64)]  # [B,N,10]

    # feature permutation: kernel row c*128+j -> ref feature c*128+2j (sin),
    # row c*128+64+j -> c*128+2j+1 (cos)
    perm = np.empty(3 * F, np.int64)
    for c in range(3):
        for j in range(64):
            perm[c * 128 + j] = c * 128 + 2 * j
            perm[c * 128 + 64 + j] = c * 128 + 2 * j + 1
    w1p = np.ascontiguousarray(np.asarray(W1, np.float32)[:, perm].T)  # [384,512]
    w2t = np.ascontiguousarray(np.asarray(W2, np.float32).T)           # [512,256]
    lwb = np.asarray(label_weight, np.float32) + np.asarray(b2, np.float32)[None]
    b1c = np.ascontiguousarray(np.asarray(b1, np.float32).reshape(4, 128).T)

    j64 = np.arange(64, dtype=np.float64)
    s64 = 10000.0 ** (-j64 / 64.0)
    s128 = np.concatenate([s64, s64])
    b128 = np.concatenate([np.zeros(64), np.full(64, np.pi / 2)])
    inv = 2 * np.pi / s128                                         # f64
    ivhi = inv.astype(np.float32).view(np.uint32) & np.uint32(0xFFFFE000)
    ivhi = ivhi.view(np.float32)          # 10 explicit mantissa bits: k*ivhi exact
    ivmid = (inv - ivhi.astype(np.float64)).astype(np.float32)

    def col(v):
        return np.ascontiguousarray(v.astype(np.float32).reshape(128, 1))

    shared = {
        "w1t": w1p.astype(BF16_NP).reshape(3, 128, 512),
        "w2t": w2t.astype(BF16_NP).reshape(4, 128, 256),
        "lwb": np.ascontiguousarray(lwb.astype(BF16_NP)),
        "svec": col(s128),
        "bvec": col(b128),
        "sdiv": col(s128 / (2 * np.pi)),
        "bdiv": col(b128 / (2 * np.pi)),
        "ivhi": col(ivhi),
        "ivmid": col(ivmid),
        "b1c": b1c,
    }

    in_maps = []
    for core in range(NCORES):
        g = core // 2
        b0 = 4 * (core % 2)
        # [4b, N, 3] -> [3, NPTS] -> [3, NT, T] -> [NT, 3, T]
        pcc = pcs[g, b0 : b0 + 4].reshape(NPTS, 3).T
        pcc = np.ascontiguousarray(pcc.reshape(3, NT, T).transpose(1, 0, 2)).reshape(
            NT, 1, 3, T
        )
        ohc = onehot[b0 : b0 + 4].reshape(NPTS, 10).T
        ohc = np.ascontiguousarray(
            ohc.reshape(10, NT, T).transpose(1, 0, 2).astype(BF16_NP)
        )
        in_maps.append({"pc": pcc, "oh": ohc, **shared})
    return in_maps


def _get_nc():
    if "nc" not in _CACHE:
        _CACHE["nc"] = _build_program()
    return _CACHE["nc"]


def _run_device(in_maps, trace=False, **kw):
    nc = _get_nc()
    return run_bass_kernel_spmd(nc, in_maps, list(range(NCORES)), trace=trace, **kw)


def kernel(point_coord, labels, pc_range, noise, query_pos, label_weight, W1, b1, W2, b2):
    in_maps = _host_prep(
        point_coord, labels, pc_range, noise, label_weight, W1, b1, W2, b2
    )
    res = _run_device(in_maps)

    qp = np.asarray(query_pos, np.float32)
    out = np.empty((G * B, N, 4 * F), np.float32)
    out[:, :, : 2 * F] = qp.reshape(G * B, N, 2 * F)
    for core in range(NCORES):
        q3 = res.results[core]["q"]                      # [128, 2, NPTS]
        q = q3.transpose(1, 0, 2).reshape(2 * F, BPC, N)  # [256, 4, N]
        out[4 * core : 4 * core + 4, :, 2 * F :] = q.transpose(1, 2, 0)
    return out


# revision 14
# speedup vs baseline: 1.9372x; 1.0571x over previous
"""Trainium2 Bass kernel for nn_GroupPointEncoder.

Reference computation (G=4, B=8, N=2048, F=128):
  std = 2 or 4 per point by label class
  coords = [point_coord, (point_coord + noise*std)[1:]]           # [G,B,N,3]
  normed = (coords - low) / (high - low)
  pe     = interleaved sin/cos embedding, (y,x,z) order            # [G,B,N,384]
  h      = relu(pe @ W1.T + b1)                                    # [G,B,N,512]
  pos    = h @ W2.T + b2                                           # [G,B,N,256]
  query  = label_weight[labels] + pos
  out    = concat([query_pos, query], -1).reshape(G*B, N, 512)

Sharding: data-parallel over the G*B=32 (g,b) pairs, 4 per core, 8 cores.
Each core computes its 4*2048=8192 points' `query` half on device; the
query_pos half is a passthrough assembled on the host.

Device pipeline (feature-major, per 512-point tile):
  bc[128,3,512]   = DMA partition-broadcast of prescaled coords (f32)
  k  (int32)      = round(bc*s/2pi + b/2pi)          DVE tensor_scalar
  r               = bc - k*(2pi/s)  (Cody-Waite)     DVE custom op
  pe (bf16)       = Sin(r*s + b)                     1 ACT op, arg in [-pi,pi]
  h  (bf16)       = relu(W1p @ pe + b1)              12 bf16 matmuls + 4 ACT
  q  (f32)        = W2 @ h + onehot.T@(lab_w+b2)     10 bf16 matmuls
                    PSUM -> SBUF copies on GpSimd, 1 output DMA per tile
"""
import sys
import math

sys.path.insert(0, "/opt/trn_rl_repo")

import numpy as np
import ml_dtypes
from contextlib import ExitStack

import concourse.bass as bass
import concourse.tile as tile
from concourse import bacc, library_config, mybir
from concourse.bass_utils import run_bass_kernel_spmd

# problem constants (hardcoded per contract)
G, B, N, F = 4, 8, 2048, 128
NCORES = 8
BPC = B * G // NCORES          # 4 (g,b) pairs per core
NPTS = BPC * N                 # 8192 points per core
T = 512                        # points per tile
NT = NPTS // T                 # 16 tiles
TWO_PI = 2.0 * math.pi
F32 = mybir.dt.float32
BF16 = mybir.dt.bfloat16
I32 = mybir.dt.int32
BF16_NP = ml_dtypes.bfloat16

_CACHE = {}


def _build_program():
    nc = bacc.Bacc("TRN2", target_bir_lowering=False, debug=False, num_devices=NCORES)

    pc_d = nc.dram_tensor("pc", [NT, 1, 3, T], F32, kind="ExternalInput").ap()
    oh_d = nc.dram_tensor("oh", [NT, 10, T], BF16, kind="ExternalInput").ap()
    w1t_d = nc.dram_tensor("w1t", [3, 128, 512], BF16, kind="ExternalInput").ap()
    w2t_d = nc.dram_tensor("w2t", [4, 128, 256], BF16, kind="ExternalInput").ap()
    lwb_d = nc.dram_tensor("lwb", [10, 256], BF16, kind="ExternalInput").ap()
    svec_d = nc.dram_tensor("svec", [128, 1], F32, kind="ExternalInput").ap()
    bvec_d = nc.dram_tensor("bvec", [128, 1], F32, kind="ExternalInput").ap()
    sdiv_d = nc.dram_tensor("sdiv", [128, 1], F32, kind="ExternalInput").ap()
    bdiv_d = nc.dram_tensor("bdiv", [128, 1], F32, kind="ExternalInput").ap()
    ivhi_d = nc.dram_tensor("ivhi", [128, 1], F32, kind="ExternalInput").ap()
    ivmid_d = nc.dram_tensor("ivmid", [128, 1], F32, kind="ExternalInput").ap()
    b1c_d = nc.dram_tensor("b1c", [128, 4], F32, kind="ExternalInput").ap()
    q_d = nc.dram_tensor("q", [128, 2, NPTS], F32, kind="ExternalOutput").ap()

    with tile.TileContext(nc) as tc, ExitStack() as ctx:
        cpool = ctx.enter_context(tc.tile_pool(name="consts", bufs=1))
        wpool = ctx.enter_context(tc.tile_pool(name="weights", bufs=1))
        io = ctx.enter_context(tc.tile_pool(name="io", bufs=4))
        bcp = ctx.enter_context(tc.tile_pool(name="bcp", bufs=4))
        work = ctx.enter_context(tc.tile_pool(name="work", bufs=3))
        qsp = ctx.enter_context(tc.tile_pool(name="qsp", bufs=3))
        psum_h = ctx.enter_context(tc.tile_pool(name="ph", bufs=3, space="PSUM"))
        psum_q = ctx.enter_context(tc.tile_pool(name="pq", bufs=2, space="PSUM"))

        # input prefetch runs 2 tiles ahead; output DMA issue is skewed 1 tile
        # behind compute so the sync queue never head-of-line blocks on it
        bc_tiles, oh_tiles, out_pend = {}, {}, {}

        def _prefetch(t):
            if t >= NT:
                return
            bc_ = bcp.tile([128, 3, T], F32, tag="bc")
            nc.sync.dma_start(bc_[:], pc_d[t].to_broadcast((128, 3, T)))
            oh_ = io.tile([10, T], BF16, tag="oh_t")
            nc.sync.dma_start(oh_[:], oh_d[t])
            bc_tiles[t], oh_tiles[t] = bc_, oh_

        # first input tile goes down the queue ahead of weights/consts so the
        # phase pipeline starts immediately
        _prefetch(0)
        svec = cpool.tile([128, 1], F32)
        nc.sync.dma_start(svec[:], svec_d[:])
        bvec = cpool.tile([128, 1], F32)
        nc.sync.dma_start(bvec[:], bvec_d[:])
        sdiv = cpool.tile([128, 1], F32)
        nc.sync.dma_start(sdiv[:], sdiv_d[:])
        bdiv = cpool.tile([128, 1], F32)
        nc.sync.dma_start(bdiv[:], bdiv_d[:])
        ivhi = cpool.tile([128, 1], F32)
        nc.sync.dma_start(ivhi[:], ivhi_d[:])
        ivmid = cpool.tile([128, 1], F32)
        nc.sync.dma_start(ivmid[:], ivmid_d[:])
        b1c = cpool.tile([128, 4], F32)
        nc.sync.dma_start(b1c[:], b1c_d[:])
        lwb = cpool.tile([10, 256], BF16)
        nc.sync.dma_start(lwb[:], lwb_d[:])

        _prefetch(1)
        w1t = []
        for kk in range(3):
            w = wpool.tile([128, 512], BF16, name=f"w1t{kk}", tag=f"w1t{kk}")
            nc.sync.dma_start(w[:], w1t_d[kk])
            w1t.append(w)
        w2t = []
        for kk in range(4):
            w = wpool.tile([128, 256], BF16, name=f"w2t{kk}", tag=f"w2t{kk}")
            nc.sync.dma_start(w[:], w2t_d[kk])
            w2t.append(w)

        for t in range(NT):
            _prefetch(t + 2)
            if t - 1 in out_pend:
                nc.sync.dma_start(
                    q_d[:, :, (t - 1) * T : t * T], out_pend.pop(t - 1)[:]
                )
            bc = bc_tiles.pop(t)
            oh_t = oh_tiles.pop(t)

            # ---- stage 1: range reduction.  k = round(bc*s/2pi + b/2pi);
            # r = bc - k*(2pi/s) via Cody-Waite; then s*r + b lands in [-pi,pi]
            bcf = bc[:].rearrange("p a b -> p (a b)")
            k = work.tile([128, 3 * T], I32, tag="k")
            nc.gpsimd.tensor_scalar(
                k[:], bcf, sdiv[:], bdiv[:],
                op0=mybir.AluOpType.mult, op1=mybir.AluOpType.add,
            )
            r = work.tile([128, 3 * T], F32, tag="r")
            nc.vector.cody_waite_cascade(r[:], bcf, k[:], ivhi[:], ivmid[:], 0.0)

            # ---- stage 2: pe = sin(s*r + b)  (rows 0:64 sin, 64:128 cos)
            # 3 per-coordinate chunks so the first W1 matmul starts sooner
            pe = work.tile([128, 3, T], BF16, tag="pe")
            rv = r[:].rearrange("p (a b) -> p a b", a=3)
            for c in range(3):
                nc.scalar.activation(
                    pe[:, c, :], rv[:, c, :],
                    mybir.ActivationFunctionType.Sin,
                    bias=bvec[:], scale=svec[:],
                )

            # ---- stage 3: h = relu(W1p @ pe + b1), feature-major [4x128, T]
            # two PSUM half-tiles so next tile's matmuls never wait on relu;
            # relu m=1 runs on DVE so relu pairs complete in parallel
            h = work.tile([128, 4, T], BF16, tag="h")
            for half in range(2):
                hp = psum_h.tile([128, 2, T], F32, tag="hp")
                for m2 in range(2):
                    m = 2 * half + m2
                    for kk in range(3):
                        nc.tensor.matmul(
                            hp[:, m2, :],
                            w1t[kk][:, m * 128 : (m + 1) * 128],
                            pe[:, kk, :],
                            start=(kk == 0),
                            stop=(kk == 2),
                        )
                    if m == 1:
                        nc.vector.tensor_scalar(
                            h[:, m, :], hp[:, m2, :], b1c[:, m : m + 1], 0.0,
                            op0=mybir.AluOpType.add, op1=mybir.AluOpType.max,
                        )
                    else:
                        nc.scalar.activation(
                            h[:, m, :],
                            hp[:, m2, :],
                            mybir.ActivationFunctionType.Relu,
                            bias=b1c[:, m : m + 1],
                        )

            # ---- stage 4: q = W2 @ h + onehot-gather, feature-major [2x128, T]
            qs = qsp.tile([128, 2, T], F32, tag="qs")
            for mp in range(2):
                qp = psum_q.tile([128, T], F32, tag="qp")
                for kk in range(4):
                    nc.tensor.matmul(
                        qp[:],
                        w2t[kk][:, mp * 128 : (mp + 1) * 128],
                        h[:, kk, :],
                        start=(kk == 0),
                        stop=False,
                    )
                nc.tensor.matmul(
                    qp[:],
                    lwb[:, mp * 128 : (mp + 1) * 128],
                    oh_t[:],
                    start=False,
                    stop=True,
                )
                nc.vector.tensor_copy(qs[:, mp, :], qp[:])
            out_pend[t] = qs
        nc.sync.dma_start(q_d[:, :, (NT - 1) * T :], out_pend.pop(NT - 1)[:])

    nc.compile()
    return nc


def _host_prep(point_coord, labels, pc_range, noise, label_weight, W1, b1, W2, b2):
    """Build the per-core input maps (host-side sharding + weight prep)."""
    pc32 = np.asarray(point_coord, np.float32)
    lab = np.asarray(labels)
    noi = np.asarray(noise, np.float32)
    rng = np.asarray(pc_range, np.float32)

    small = (lab == 0) | (lab >= 6)
    std = np.where(small, 2.0, 4.0).astype(np.float32)            # [B,N]
    coords = pc32[None] + noi * std[None, :, :, None]             # [G,B,N,3]
    coords[0] = pc32                                              # group 0 originals
    low, high = rng[:3], rng[3:]
    pcs = (coords - low) / (high - low) * np.float32(TWO_PI)      # [G,B,N,3]
    pcs = pcs[..., [1, 0, 2]]   # reference concatenates pe in (y,x,z) order
    onehot = np.eye(10, dtype=np.float32)[np.asarray(lab, np.int64)]  # [B,N,10]

    # feature permutation: kernel row c*128+j -> ref feature c*128+2j (sin),
    # row c*128+64+j -> c*128+2j+1 (cos)
    perm = np.empty(3 * F, np.int64)
    for c in range(3):
        for j in range(64):
            perm[c * 128 + j] = c * 128 + 2 * j
            perm[c * 128 + 64 + j] = c * 128 + 2 * j + 1
    w1p = np.ascontiguousarray(np.asarray(W1, np.float32)[:, perm].T)  # [384,512]
    w2t = np.ascontiguousarray(np.asarray(W2, np.float32).T)           # [512,256]
    lwb = np.asarray(label_weight, np.float32) + np.asarray(b2, np.float32)[None]
    b1c = np.ascontiguousarray(np.asarray(b1, np.float32).reshape(4, 128).T)

    j64 = np.arange(64, dtype=np.float64)
    s64 = 10000.0 ** (-j64 / 64.0)
    s128 = np.concatenate([s64, s64])
    b128 = np.concatenate([np.zeros(64), np.full(64, np.pi / 2)])
    inv = 2 * np.pi / s128                                         # f64
    ivhi = inv.astype(np.float32).view(np.uint32) & np.uint32(0xFFFFE000)
    ivhi = ivhi.view(np.float32)          # 10 explicit mantissa bits: k*ivhi exact
    ivmid = (inv - ivhi.astype(np.float64)).astype(np.float32)

    def col(v):
        return np.ascontiguousarray(v.astype(np.float32).reshape(128, 1))

    shared = {
        "w1t": w1p.astype(BF16_NP).reshape(3, 128, 512),
        "w2t": w2t.astype(BF16_NP).reshape(4, 128, 256),
        "lwb": np.ascontiguousarray(lwb.astype(BF16_NP)),
        "svec": col(s128),
        "bvec": col(b128),
        "sdiv": col(s128 / (2 * np.pi)),
        "bdiv": col(b128 / (2 * np.pi)),
        "ivhi": col(ivhi),
        "ivmid": col(ivmid),
        "b1c": b1c,
    }

    in_maps = []
    for core in range(NCORES):
        g = core // 2
        b0 = 4 * (core % 2)
        # [4b, N, 3] -> [3, NPTS] -> [3, NT, T] -> [NT, 3, T]
        pcc = pcs[g, b0 : b0 + 4].reshape(NPTS, 3).T
        pcc = np.ascontiguousarray(pcc.reshape(3, NT, T).transpose(1, 0, 2)).reshape(
            NT, 1, 3, T
        )
        ohc = onehot[b0 : b0 + 4].reshape(NPTS, 10).T
        ohc = np.ascontiguousarray(
            ohc.reshape(10, NT, T).transpose(1, 0, 2).astype(BF16_NP)
        )
        in_maps.append({"pc": pcc, "oh": ohc, **shared})
    return in_maps


def _get_nc():
    if "nc" not in _CACHE:
        _CACHE["nc"] = _build_program()
    return _CACHE["nc"]


def _run_device(in_maps, trace=False, **kw):
    nc = _get_nc()
    return run_bass_kernel_spmd(nc, in_maps, list(range(NCORES)), trace=trace, **kw)


def kernel(point_coord, labels, pc_range, noise, query_pos, label_weight, W1, b1, W2, b2):
    in_maps = _host_prep(
        point_coord, labels, pc_range, noise, label_weight, W1, b1, W2, b2
    )
    res = _run_device(in_maps)

    qp = np.asarray(query_pos, np.float32)
    out = np.empty((G * B, N, 4 * F), np.float32)
    out[:, :, : 2 * F] = qp.reshape(G * B, N, 2 * F)
    for core in range(NCORES):
        q3 = res.results[core]["q"]                      # [128, 2, NPTS]
        q = q3.transpose(1, 0, 2).reshape(2 * F, BPC, N)  # [256, 4, N]
        out[4 * core : 4 * core + 4, :, 2 * F :] = q.transpose(1, 2, 0)
    return out
